# revision 1
# baseline (speedup 1.0000x reference)
"""Trainium2 Bass kernel for a dense transformer block (pre-LN, causal MHA + FF).

Reference semantics (fp32, per batch row b of 2048, seq T=64, embd C=256):
    h   = LN(x; g1, be1)
    q,k,v = per-head projections (16 heads x 32 dims)
    att = softmax(causal(q k^T / sqrt(32))) v        -> concat heads
    x2  = x + att @ Wp + bp
    out = x2 + relu(LN(x2; g2, be2) @ W1 + b1) @ W2 + b2

Strategy: pure data parallel over 8 NeuronCores (256 batch rows each).
Per core: 32 j-blocks of 512 tokens (4 tiles of 128 tokens = 2 batch rows).
All dense projections stream N=512 moving columns through the PE to amortize
instruction overhead and keep the HAM clock-gate warm; attention uses the
2-chain prefix trick (PE operand partition bases limited to {0,64}) with
tile_position packing; token-major o with a ones-column denominator; engine
load spread across DVE / Scalar / Pool.
"""

import os
import sys

sys.path.insert(0, "/opt/trn_rl_repo")

import numpy as np
import concourse.bass as bass
import concourse.mybir as mybir
import concourse.tile as tile
from concourse.bass_utils import run_bass_kernel_spmd

# ---------------------------------------------------------------- constants
N_CORES = 8
N_EMBD = 256
N_HEAD = 16
HEAD = 32
T = 64
BATCH = 2048
B_LOC = BATCH // N_CORES          # 256 batch rows per core
TOK = B_LOC * T                   # 16384 tokens per core
P = 128                           # tokens per tile (2 batch rows)
JT = 4                            # tiles per j-block
JTOK = P * JT                     # 512 tokens per j-block
NJB = TOK // JTOK                 # 32 j-blocks per core
SCALE = 1.0 / np.sqrt(HEAD)
EPS = 1e-5

FDT = mybir.dt.float32
CDT = mybir.dt.bfloat16
AFT = mybir.ActivationFunctionType

_MAX_DRAIN_WAITS = 1


def _split_waits(nc, limit=1):
    """walrus in this build encodes only `limit` sync waits per CTRL/compute
    instruction; move overflow waits onto preceding same-engine NOPs."""
    n = 0
    for f in nc.m.functions:
        for bb in f.blocks:
            insts = bb.instructions
            i = 0
            while i < len(insts):
                inst = insts[i]
                si = getattr(inst, "sync_info", None)
                if si is not None and si.on_wait and len(si.on_wait) > limit:
                    waits = list(si.on_wait)
                    keep, extra = waits[:limit], waits[limit:]
                    inst.sync_info = mybir.SyncInfo(
                        on_wait=keep, on_update=list(si.on_update or [])
                    )
                    for j, w in enumerate(extra):
                        nop = mybir.InstNoOp(
                            name=f"{inst.name}-wsplit{j}",
                            ins=[], outs=[],
                            engine=inst.engine,
                            bass_nofuse=True,
                            sync_info=mybir.SyncInfo(on_wait=[w], on_update=[]),
                        )
                        nc.register_instruction(nop, overwrite=True)
                        insts.insert(i, nop)
                        i += 1
                        n += 1
                i += 1
    return n


# ---------------------------------------------------------------- program
def build_program(flags, njb=NJB):
    """flags: (has_bq, has_bk, has_bv, has_bp, has_b1, has_b2)."""
    has_bq, has_bk, has_bv, has_bp, has_b1, has_b2 = flags
    nc = bass.Bass()

    x_d = nc.declare_dram_parameter("x", [njb * JTOK, N_EMBD], FDT, isOutput=False)
    wq_d = nc.declare_dram_parameter("wq", [128, 1024], CDT, isOutput=False)
    wk_d = nc.declare_dram_parameter("wk", [128, 1024], CDT, isOutput=False)
    wv_d = nc.declare_dram_parameter("wv", [128, 1024], CDT, isOutput=False)
    wp_d = nc.declare_dram_parameter("wp", [128, 1024], CDT, isOutput=False)
    w1_d = nc.declare_dram_parameter("w1", [128, 2048], CDT, isOutput=False)
    w2_d = nc.declare_dram_parameter("w2", [128, 2048], CDT, isOutput=False)
    id_d = nc.declare_dram_parameter("ident", [128, 128], CDT, isOutput=False)
    mk_d = nc.declare_dram_parameter("cmask", [128, T], CDT, isOutput=False)
    bq_d = bk_d = bv_d = bp_d = b1_d = b2_d = None
    if has_bq:
        bq_d = nc.declare_dram_parameter("bq", [128, 4], FDT, isOutput=False)
    if has_bk:
        bk_d = nc.declare_dram_parameter("bk", [128, 4], FDT, isOutput=False)
    if has_bv:
        bv_d = nc.declare_dram_parameter("bv", [128, 16 * 33], FDT, isOutput=False)
    if has_bp:
        bp_d = nc.declare_dram_parameter("bp", [128, N_EMBD], FDT, isOutput=False)
    if has_b1:
        b1_d = nc.declare_dram_parameter("b1", [128, 8], FDT, isOutput=False)
    if has_b2:
        b2_d = nc.declare_dram_parameter("b2", [128, N_EMBD], FDT, isOutput=False)
    out_d = nc.declare_dram_parameter("out", [njb * JTOK, N_EMBD], FDT, isOutput=True)

    with tile.TileContext(nc, linearize=bool(os.environ.get('KLIN'))) as tc:
        with (
            tc.tile_pool(name="consts", bufs=1) as cpool,
            tc.tile_pool(name="work", bufs=2) as wpool,
            tc.tile_pool(name="psum", bufs=1, space="PSUM") as ppool,
        ):
            wq_t = cpool.tile([128, 1024], CDT)
            wk_t = cpool.tile([128, 1024], CDT)
            wv_t = cpool.tile([128, 1024], CDT)
            wp_t = cpool.tile([128, 1024], CDT)
            w1_t = cpool.tile([128, 2048], CDT)
            w2_t = cpool.tile([128, 2048], CDT)
            ident = cpool.tile([128, 128], CDT)
            cmask = cpool.tile([128, T], CDT)
            eps_sb = cpool.tile([128, 1], FDT)
            nc.gpsimd.memset(eps_sb[:], EPS)
            for t_, d_ in [(wq_t, wq_d), (wk_t, wk_d), (wv_t, wv_d), (wp_t, wp_d),
                           (w1_t, w1_d), (w2_t, w2_d), (ident, id_d), (cmask, mk_d)]:
                nc.sync.dma_start(t_[:], d_[:])
            wq = wq_t[:].rearrange("p (kk m j) -> p kk m j", kk=2, m=4)
            wk = wk_t[:].rearrange("p (kk m j) -> p kk m j", kk=2, m=4)
            wv = wv_t[:].rearrange("p (kk n) -> p kk n", kk=2)
            wp = wp_t[:].rearrange("p (f n) -> p f n", f=4)
            w1 = w1_t[:].rearrange("p (kk m j) -> p kk m j", kk=2, m=8)
            w2 = w2_t[:].rearrange("p (f n) -> p f n", f=8)
            bq = bk = bv = bpB = b1 = b2B = None
            if has_bq:
                bq = cpool.tile([128, 4], FDT)
                nc.sync.dma_start(bq[:], bq_d[:])
            if has_bk:
                bk = cpool.tile([128, 4], FDT)
                nc.sync.dma_start(bk[:], bk_d[:])
            if has_bv:
                bv = cpool.tile([128, 16 * 33], FDT)
                nc.sync.dma_start(bv[:], bv_d[:])
            if has_bp:
                bpB = cpool.tile([128, N_EMBD], FDT)
                nc.sync.dma_start(bpB[:], bp_d[:])
            if has_b1:
                b1 = cpool.tile([128, 8], FDT)
                nc.sync.dma_start(b1[:], b1_d[:])
            if has_b2:
                b2B = cpool.tile([128, N_EMBD], FDT)
                nc.sync.dma_start(b2B[:], b2_d[:])

            def layernorm(dst_cdt, src_f32):
                """src [128, 4, 256] fp32 -> dst [128, 4, 256] bf16 normalized."""
                st = wpool.tile([128, 4, 6], FDT, tag="st")
                for j in range(JT):
                    nc.vector.bn_stats(st[:, j, :], src_f32[:, j, :])
                mv = wpool.tile([128, 4, 2], FDT, tag="mv")
                for j in range(JT):
                    nc.vector.bn_aggr(mv[:, j, :], st[:, j, :])
                lnv = wpool.tile([128, 4], FDT, tag="lnv")
                nc.scalar.activation(lnv[:], mv[:, :, 1], AFT.Ln, bias=eps_sb[:])
                rstd = wpool.tile([128, 4], FDT, tag="rstd")
                nc.scalar.activation(rstd[:], lnv[:], AFT.Exp, scale=-0.5)
                for j in range(JT):
                    nc.vector.tensor_scalar(
                        dst_cdt[:, j, :], src_f32[:, j, :],
                        mv[:, j, 0:1], rstd[:, j:j + 1],
                        mybir.AluOpType.subtract, mybir.AluOpType.mult,
                    )

            def transpose2(dst, src_cdt):
                """src [128, 4, 256] bf16 -> dst [128, 2, 512] bf16 (feature-major)."""
                for kk in range(2):
                    tr_ps = ppool.tile([128, 512], CDT, tag="tr", bufs=1)
                    for j in range(JT):
                        nc.tensor.transpose(
                            tr_ps[:, j * 128:(j + 1) * 128],
                            src_cdt[:, j, kk * 128:(kk + 1) * 128], ident[:],
                        )
                    nc.scalar.copy(dst[:, kk, :], tr_ps[:])

            for jb in range(njb):
                rows = slice(jb * JTOK, (jb + 1) * JTOK)

                # ---- load x j-block (token-major [128 tok, 4 j, 256 c])
                x_sb = wpool.tile([128, JT, N_EMBD], FDT)
                nc.sync.dma_start(
                    x_sb[:], x_d[rows, :].rearrange("(j p) c -> p j c", p=128)
                )

                # ---- LN1
                xhat = wpool.tile([128, JT, N_EMBD], CDT)
                layernorm(xhat, x_sb)
                xhatT = wpool.tile([128, 2, 512], CDT)
                transpose2(xhatT, xhat)

                # ---- qT/kT feature-major [hd, tok]: chunk m holds heads 4m..4m+3
                qT = wpool.tile([128, 4, 512], CDT)
                kT = wpool.tile([128, 4, 512], CDT)
                for dstT, w_, b_, has_, eng in (
                    (qT, wq, bq, has_bq, nc.scalar),
                    (kT, wk, bk, has_bk, nc.gpsimd),
                ):
                    for m in range(4):
                        mm_ps = ppool.tile([128, 512], FDT, tag="mm", bufs=2)
                        for kk in range(2):
                            nc.tensor.matmul(
                                mm_ps[:], w_[:, kk, m, :], xhatT[:, kk, :],
                                start=(kk == 0), stop=(kk == 1),
                            )
                        if eng is nc.scalar:
                            nc.scalar.copy(dstT[:, m, :], mm_ps[:])
                        else:
                            nc.vector.tensor_copy(dstT[:, m, :], mm_ps[:])
                        if has_:
                            nc.vector.tensor_scalar_add(
                                dstT[:, m, :], dstT[:, m, :], b_[:, m:m + 1]
                            )

                # ---- v token-major [tok, h, 33] with ones column at d=32
                v_sb = wpool.tile([128, JT, 16, 33], CDT)
                for it in range(JT):
                    v_ps = ppool.tile([128, 512], FDT, tag="mm", bufs=2)
                    for kk in range(2):
                        nc.tensor.matmul(
                            v_ps[:], xhatT[:, kk, it * 128:(it + 1) * 128],
                            wv[:, kk, :],
                            start=(kk == 0), stop=(kk == 1),
                        )
                    nc.scalar.copy(
                        v_sb[:, it, :, 0:32],
                        v_ps[:].rearrange("p (h d) -> p h d", h=16),
                    )
                    nc.gpsimd.memset(v_sb[:, it, :, 32:33], 1.0)
                    if has_bv:
                        nc.vector.scalar_tensor_tensor(
                            v_sb[:, it, :, 0:32], v_sb[:, it, :, 0:32], 1.0,
                            bv[:].rearrange("p (h d) -> p h d", h=16)[:, :, 0:32],
                            op0=mybir.AluOpType.mult, op1=mybir.AluOpType.add,
                        )

                # ---- attention + proj + residual per tile
                x2_sb = wpool.tile([128, JT, N_EMBD], FDT)
                for it in range(JT):
                    # scores, 2-chain prefix (PE reads only at part base 0/64).
                    # col layout [par, g, e, t]: par=0 direct score of head
                    # 4g+2e (K=32), par=1 cumulative 4g+2e..+1 (K=64); chain
                    # for heads (4g, 4g+1) reads kT/qT rows 0:64, heads
                    # (4g+2, 4g+3) rows 64:128.
                    sc_ps = ppool.tile([128, 2, 4, 2, T], FDT, tag="sc", bufs=1)
                    for g in range(4):
                        for e in range(2):
                            for par, kd in ((0, 32), (1, 64)):
                                for b in range(2):
                                    cols = slice(it * 128 + b * 64,
                                                 it * 128 + (b + 1) * 64)
                                    nc.tensor.matmul(
                                        sc_ps[b * 64:(b + 1) * 64, par, g, e, :],
                                        kT[64 * e:64 * e + kd, g, cols],
                                        qT[64 * e:64 * e + kd, g, cols],
                                        tile_position=(64 * e, b * 64),
                                    )
                    # unstack: cum - direct = odd-head scores (via sbuf)
                    scS = wpool.tile([128, 4, 2, T], FDT, tag="scS")
                    nc.vector.tensor_copy(scS[:], sc_ps[:, 1, :, :, :])
                    nc.vector.tensor_tensor(
                        scS[:], scS[:], sc_ps[:, 0, :, :, :],
                        mybir.AluOpType.subtract,
                    )
                    # exp (scaled) -> bf16, then causal mask (multiplicative)
                    expT = wpool.tile([128, 2, 4, 2, T], CDT, tag="expT")
                    nc.scalar.activation(
                        expT[:, 0, :, :, :], sc_ps[:, 0, :, :, :],
                        AFT.Exp, scale=float(SCALE),
                    )
                    nc.scalar.activation(
                        expT[:, 1, :, :, :], scS[:], AFT.Exp, scale=float(SCALE),
                    )
                    e_view = expT[:].rearrange("p a g e t -> p (a g e) t")
                    e_b, mk_b = bass.broadcast_tensor_aps(
                        e_view, cmask[:].rearrange("p (o s) -> p o s", o=1)
                    )
                    nc.vector.tensor_tensor(
                        e_view, e_b, mk_b, mybir.AluOpType.mult
                    )
                    # o token-major [t, h, 33] (col 32 = softmax denominator);
                    # two 8-head psum tiles: [128,16,33] would cross a bank
                    o_hps = [ppool.tile([128, 8, 33], FDT, tag=f"o{half}",
                                        bufs=1, name=f"o_ps{half}")
                             for half in range(2)]
                    for g in range(4):
                        for hp in range(4):
                            h = 4 * g + hp
                            for b in range(2):
                                nc.tensor.matmul(
                                    o_hps[h // 8][b * 64:(b + 1) * 64, h % 8, :],
                                    expT[b * 64:(b + 1) * 64, hp % 2, g, hp // 2, :],
                                    v_sb[b * 64:(b + 1) * 64, it, h, :],
                                    tile_position=(b * 64, b * 64),
                                )
                    rec = wpool.tile([128, 16], FDT, tag="rec")
                    o_sb = wpool.tile([128, 16, 32], CDT, tag="o_sb")
                    for half in range(2):
                        hs = slice(half * 8, (half + 1) * 8)
                        nc.vector.reciprocal(rec[:, hs], o_hps[half][:, :, 32:33])
                        o_num_b, rec_b = bass.broadcast_tensor_aps(
                            o_hps[half][:, :, 0:32],
                            rec[:, hs].rearrange("p (h o) -> p h o", o=1),
                        )
                        nc.vector.tensor_tensor(
                            o_sb[:, hs, :], o_num_b, rec_b, mybir.AluOpType.mult
                        )
                    if os.environ.get("KTAP") in ("o0", "o1"):
                        half = 0 if os.environ["KTAP"] == "o0" else 1
                        nc.vector.tensor_copy(
                            x2_sb[:, it, :],
                            o_sb[:].rearrange("p h d -> p (h d)")[:, half * 256:(half + 1) * 256],
                        )
                        continue
                    # oT feature-major via PE transpose
                    to_ps = ppool.tile([128, 512], CDT, tag="tr", bufs=1)
                    o_flat = o_sb[:].rearrange("p h d -> p (h d)")
                    for f in range(4):
                        nc.tensor.transpose(
                            to_ps[:, f * 128:(f + 1) * 128],
                            o_flat[:, f * 128:(f + 1) * 128], ident[:],
                        )
                    oT = wpool.tile([128, 512], CDT, tag="oT")
                    nc.vector.tensor_copy(oT[:], to_ps[:])
                    # proj: sa = o @ Wp (token-major out)
                    sa_ps = ppool.tile([128, N_EMBD], FDT, tag="sa", bufs=1)
                    for f in range(4):
                        nc.tensor.matmul(
                            sa_ps[:], oT[:, f * 128:(f + 1) * 128], wp[:, f, :],
                            start=(f == 0), stop=(f == 3),
                        )
                    # residual 1
                    if has_bp:
                        nc.vector.scalar_tensor_tensor(
                            x2_sb[:, it, :], sa_ps[:], 1.0, bpB[:],
                            op0=mybir.AluOpType.mult, op1=mybir.AluOpType.add,
                        )
                        nc.vector.tensor_add(
                            x2_sb[:, it, :], x2_sb[:, it, :], x_sb[:, it, :]
                        )
                    else:
                        nc.vector.tensor_tensor(
                            x2_sb[:, it, :], sa_ps[:], x_sb[:, it, :],
                            mybir.AluOpType.add,
                        )

                # ---- LN2
                xh2 = wpool.tile([128, JT, N_EMBD], CDT)
                layernorm(xh2, x2_sb)
                xh2T = wpool.tile([128, 2, 512], CDT)
                transpose2(xh2T, xh2)

                # ---- FF1 feature-major: rT chunk m = [128 ff, 512 tok]
                rT = wpool.tile([128, 8, 512], CDT)
                for m in range(8):
                    f1_ps = ppool.tile([128, 512], FDT, tag="mm", bufs=2)
                    for kk in range(2):
                        nc.tensor.matmul(
                            f1_ps[:], w1[:, kk, m, :], xh2T[:, kk, :],
                            start=(kk == 0), stop=(kk == 1),
                        )
                    if has_b1:
                        nc.scalar.activation(
                            rT[:, m, :], f1_ps[:], AFT.Relu, bias=b1[:, m:m + 1]
                        )
                    else:
                        nc.scalar.activation(rT[:, m, :], f1_ps[:], AFT.Relu)

                # ---- FF2 token-major + residual 2 + store
                out_sb = wpool.tile([128, JT, N_EMBD], FDT)
                for it in range(JT):
                    ff_ps = ppool.tile([128, N_EMBD], FDT, tag="sa", bufs=1)
                    for f in range(8):
                        nc.tensor.matmul(
                            ff_ps[:], rT[:, f, it * 128:(it + 1) * 128],
                            w2[:, f, :],
                            start=(f == 0), stop=(f == 7),
                        )
                    nc.vector.tensor_tensor(
                        out_sb[:, it, :], ff_ps[:], x2_sb[:, it, :],
                        mybir.AluOpType.add,
                    )
                    if has_b2:
                        nc.vector.tensor_add(
                            out_sb[:, it, :], out_sb[:, it, :], b2B[:]
                        )
                tap = os.environ.get("KTAP")
                if tap == "xhat":
                    nc.vector.tensor_copy(out_sb[:], xhat[:])
                elif tap in ("x2", "o0", "o1"):
                    nc.vector.tensor_copy(out_sb[:], x2_sb[:])
                elif tap == "xh2":
                    nc.vector.tensor_copy(out_sb[:], xh2[:])
                nc.sync.dma_start(
                    out_d[rows, :].rearrange("(j p) c -> p j c", p=128), out_sb[:]
                )

    _split_waits(nc)
    nc.finalize()
    return nc


# ---------------------------------------------------------------- host prep
def _prep_weights(Wq, Wk, Wv, Wp, bp, W1, b1, W2, b2, g1, be1, g2, be2):
    import ml_dtypes

    cdt = ml_dtypes.bfloat16
    g1 = g1.astype(np.float32); be1 = be1.astype(np.float32)
    g2 = g2.astype(np.float32); be2 = be2.astype(np.float32)

    def lhsT_layout(W, n_k, n_m):  # W [K, M] -> [128, n_k * n_m * 128]
        return (
            W.reshape(n_k, 128, n_m, 128).transpose(1, 0, 2, 3).reshape(128, -1)
        )

    def rhs_layout(W, n_k):  # W [K, N] -> [128, n_k * N]
        K, N = W.shape
        return W.reshape(n_k, 128, N).transpose(1, 0, 2).reshape(128, -1)

    Wqf = (g1[:, None] * Wq.transpose(1, 0, 2).reshape(N_EMBD, 512)).astype(np.float32)
    Wkf = (g1[:, None] * Wk.transpose(1, 0, 2).reshape(N_EMBD, 512)).astype(np.float32)
    Wvf = (g1[:, None] * Wv.transpose(1, 0, 2).reshape(N_EMBD, 512)).astype(np.float32)
    bqv = be1 @ Wqf
    bkv = be1 @ Wkf
    bvv = be1 @ Wvf
    W1f = (g2[:, None] * W1).astype(np.float32)
    b1f = b1.astype(np.float32) + be2 @ W1f

    inp = {
        "wq": lhsT_layout(Wqf, 2, 4).astype(cdt),
        "wk": lhsT_layout(Wkf, 2, 4).astype(cdt),
        "wv": rhs_layout(Wvf, 2).astype(cdt),
        "wp": rhs_layout(Wp.astype(np.float32), 4).astype(cdt),
        "w1": lhsT_layout(W1f, 2, 8).astype(cdt),
        "w2": rhs_layout(W2.astype(np.float32), 8).astype(cdt),
        "ident": np.eye(128, dtype=np.float32).astype(cdt),
        "cmask": np.tile(
            (np.arange(T)[:, None] <= np.arange(T)[None, :]), (2, 1)
        ).astype(np.float32).astype(cdt),
    }
    flags = (
        bool(np.any(bqv)), bool(np.any(bkv)), bool(np.any(bvv)),
        bool(np.any(bp)), bool(np.any(b1f)), bool(np.any(b2)),
    )
    if flags[0]:
        inp["bq"] = bqv.reshape(4, 128).T.astype(np.float32).copy()
    if flags[1]:
        inp["bk"] = bkv.reshape(4, 128).T.astype(np.float32).copy()
    if flags[2]:
        bvt = np.zeros((128, 16 * 33), np.float32)
        for h in range(N_HEAD):
            bvt[:, h * 33: h * 33 + 32] = bvv[h * 32:(h + 1) * 32][None, :]
        inp["bv"] = bvt
    if flags[3]:
        inp["bp"] = np.tile(bp.astype(np.float32)[None, :], (128, 1))
    if flags[4]:
        inp["b1"] = b1f.reshape(8, 128).T.astype(np.float32).copy()
    if flags[5]:
        inp["b2"] = np.tile(b2.astype(np.float32)[None, :], (128, 1))
    return inp, flags


_prog_cache = {}


def _get_program(flags, njb=NJB):
    key = (flags, njb)
    if key not in _prog_cache:
        _prog_cache[key] = build_program(flags, njb)
    return _prog_cache[key]


def run(inputs, njb=NJB, n_cores=N_CORES, trace=False):
    """inputs: full-size dict as from setup_inputs(). Returns (out, results)."""
    x = np.asarray(inputs["x"], dtype=np.float32)
    B = x.shape[0]
    winp, flags = _prep_weights(
        *(np.asarray(inputs[k]) for k in
          ["Wq", "Wk", "Wv", "Wp", "bp", "W1", "b1", "W2", "b2",
           "g1", "be1", "g2", "be2"])
    )
    nc = _get_program(flags, njb)
    b_loc = B // n_cores
    shards = x.reshape(n_cores, b_loc * T, N_EMBD)
    in_maps = [dict(winp, x=np.ascontiguousarray(shards[i])) for i in range(n_cores)]
    res = run_bass_kernel_spmd(
        nc, in_maps, core_ids=list(range(n_cores)), trace=trace
    )
    out = np.concatenate(
        [res.results[i]["out"].reshape(b_loc, T, N_EMBD) for i in range(n_cores)],
        axis=0,
    )
    return out.astype(np.float32), res


def kernel(**inputs):
    out, _ = run(inputs)
    return out

